# revision 9
# baseline (speedup 1.0000x reference)
"""Trainium2 Bass kernel for CriterionIFV (segment-reduce / class-center cosine distill loss).

Math (per sample b, all labels in [0, 19)):
    S[c,k]   = sum_{p: lab[p]=k} feat[c,p]          (segment sum, both features)
    n[k]     = |{p: lab[p]=k}|
    M[c,k]   = S[c,k] / (n[k] + 1e-6)
    Mhat     = M * (1 / max(|M[:,k]|, 1e-8))        (column-normalized means)
    G[p,k]   = sum_c feat[c,p] * Mhat[c,k]
    dot[p]   = G[p, lab[p]]
    cos[p]   = dot[p] / max(|feat[:,p]|, 1e-8)
    out      = mean_p (cos_S[p] - cos_T[p])^2       (global mean over B*H*W)

Sharding: data-parallel over batch B=8 across the 8 NeuronCores (1 sample each).
Each core returns its partial sum of squared diffs; host combines (the final
"all-reduce" of a single scalar) and divides by B*H*W.

Two streaming passes over the features per core:
  pass 1: f32->bf16 cast-loads (SWDGE), DMA-xbar transpose to pixel-major
          tiles, PE segment-sum matmuls (onehot^T stationary), fused DVE
          square+reduce for per-pixel norms.
  pass 2: f32->bf16 cast-loads, PE per-pixel-chunk matmuls against Mhat
          (G^T orientation, pixels on partitions), DVE onehot-select + cosine
          + squared-diff accumulation.
"""

import numpy as np
from contextlib import ExitStack

# ---- problem constants (hardcoded; kernel.py must be self-contained) ----
B = 8
C = 512
H = W = 128
HW = H * W            # 16384 pixels per sample
K = 19                # num classes
P = 128               # partitions
CC = C // P           # 4 channel chunks
NCH = HW // P         # 128 pixel chunks of 128
WPIX = 1024           # pixels per load window
NW = HW // WPIX       # 16 windows
CHW = WPIX // P       # 8 chunks per window
EPS_MEAN = 1e-6
EPS_COS = 1e-8

_CACHE = {}
TRACE = False         # set True from test harness to capture an NTFF profile
LAST_RESULTS = None   # BassKernelResults of the most recent run (for profiling)


def _build_nc():
    import concourse.bacc as bacc
    import concourse.bass as bass
    import concourse.tile as tile
    from concourse import mybir
    from concourse.masks import make_identity

    f32 = mybir.dt.float32
    bf16 = mybir.dt.bfloat16
    i32 = mybir.dt.int32
    Alu = mybir.AluOpType
    Act = mybir.ActivationFunctionType

    nc = bacc.Bacc("TRN2", target_bir_lowering=False, debug=False)

    xs = nc.dram_tensor("xs", [C, HW], f32, kind="ExternalInput")
    xt = nc.dram_tensor("xt", [C, HW], f32, kind="ExternalInput")
    # labT[i, ch] = labels[ch*128 + i]  (host pre-transposed, as float32)
    labT = nc.dram_tensor("labT", [P, NCH], f32, kind="ExternalInput")
    o = nc.dram_tensor("o", [1, 1], f32, kind="ExternalOutput")

    with tile.TileContext(nc) as tc, ExitStack() as ctx:
        singles = ctx.enter_context(tc.tile_pool(name="singles", bufs=1))
        nat = ctx.enter_context(tc.tile_pool(name="nat", bufs=2))
        ftp = ctx.enter_context(tc.tile_pool(name="ftp", bufs=3))
        dvetmp = ctx.enter_context(tc.tile_pool(name="dvetmp", bufs=2))
        small = ctx.enter_context(tc.tile_pool(name="small", bufs=2))

        # ---------------- setup ----------------
        labT_sb = singles.tile([P, NCH], f32)
        nc.sync.dma_start(out=labT_sb, in_=labT[:, :])

        iota_i = singles.tile([P, K], i32)
        nc.gpsimd.iota(iota_i, [[1, K]], base=0, channel_multiplier=0)
        iota_f = singles.tile([P, K], f32)
        nc.vector.tensor_copy(iota_f, iota_i)

        ones_bf = singles.tile([P, 1], bf16)
        nc.vector.memset(ones_bf, 1.0)
        ones_f = singles.tile([P, 1], f32)
        nc.vector.memset(ones_f, 1.0)

        ident19 = singles.tile([K, K], f32)
        make_identity(nc, ident19)

        ohT_map = singles.tile([P, NCH * K], bf16)      # onehot^T per chunk
        fnsq = {fn: singles.tile([P, NCH], f32, name=f"fnsq_{fn}") for fn in "st"}
        invfn = {fn: singles.tile([P, NCH], f32, name=f"invfn_{fn}") for fn in "st"}

        with tc.tile_pool(name="psum1", bufs=1, space="PSUM") as psum1:
            ps_S = {fn: psum1.tile([K, C], f32, tag=f"ps_{fn}", name=f"ps_{fn}")
                    for fn in "st"}
            ps_N = psum1.tile([K, 1], f32, tag="ps_n")

            # ---------------- pass 1 ----------------
            for w in range(NW):
                nats = {}
                for fn, x in (("s", xs), ("t", xt)):
                    for cc in range(CC):
                        t = nat.tile([P, WPIX], bf16, tag=f"nat_{fn}{cc}")
                        nc.gpsimd.dma_start(
                            out=t,
                            in_=x[cc * P:(cc + 1) * P, w * WPIX:(w + 1) * WPIX],
                        )
                        nats[fn, cc] = t
                for j in range(CHW):
                    ch = w * CHW + j
                    first, last = (ch == 0), (ch == NCH - 1)
                    oh = ohT_map[:, ch * K:(ch + 1) * K]
                    nc.vector.tensor_scalar(
                        out=oh, in0=iota_f, scalar1=labT_sb[:, ch:ch + 1],
                        scalar2=None, op0=Alu.is_equal,
                    )
                    ft = {}
                    for fi, fn in enumerate("st"):
                        t = ftp.tile([P, C], bf16, tag=f"ft_{fn}")
                        for cc in range(CC):
                            eng = nc.sync if (cc + fi) % 2 == 0 else nc.scalar
                            eng.dma_start(
                                out=t[:, cc * P:(cc + 1) * P],
                                in_=nats[fn, cc][:, j * P:(j + 1) * P],
                                transpose=True,
                            )
                        ft[fn] = t
                    for fn in "st":
                        nc.tensor.matmul(ps_S[fn], oh, ft[fn], start=first, stop=last)
                        sq = dvetmp.tile([P, C], bf16, tag="ttr_sq")
                        nc.vector.tensor_mul(sq, ft[fn], ft[fn])
                        nc.vector.tensor_reduce(
                            out=fnsq[fn][:, ch:ch + 1], in_=sq,
                            axis=mybir.AxisListType.X, op=Alu.add,
                        )
                    nc.tensor.matmul(ps_N, oh, ones_bf, start=first, stop=last)

            # ---------------- class means ----------------
            inv_n = small.tile([K, 1], f32, tag="inv_n")
            nc.vector.tensor_scalar(out=inv_n, in0=ps_N, scalar1=EPS_MEAN,
                                    scalar2=None, op0=Alu.add)
            inv_n2 = small.tile([K, 1], f32, tag="inv_n2")
            nc.vector.reciprocal(inv_n2, inv_n)

            mh = {}  # mh[fn][cc]: [128, K] bf16 column-normalized means
            with tc.tile_pool(name="psum_tr", bufs=2, space="PSUM") as psum_tr:
                for fn in "st":
                    mt = small.tile([K, C], f32, tag=f"mt_{fn}")
                    nc.vector.tensor_scalar(out=mt, in0=ps_S[fn], scalar1=inv_n2,
                                            scalar2=None, op0=Alu.mult)
                    mnsq = small.tile([K, 1], f32, tag=f"mnsq_{fn}")
                    mdum = dvetmp.tile([K, C], f32, tag="mdum")
                    nc.vector.tensor_mul(mdum, mt, mt)
                    nc.vector.tensor_reduce(
                        out=mnsq, in_=mdum, axis=mybir.AxisListType.X, op=Alu.add,
                    )
                    mn = small.tile([K, 1], f32, tag=f"mn_{fn}")
                    nc.scalar.activation(out=mn, in_=mnsq, func=Act.Sqrt)
                    nc.vector.tensor_scalar_max(mn, mn, EPS_COS)
                    invmn = small.tile([K, 1], f32, tag=f"invmn_{fn}")
                    nc.vector.reciprocal(invmn, mn)
                    mhT = small.tile([K, C], f32, tag=f"mhT_{fn}")
                    nc.vector.tensor_scalar(out=mhT, in0=mt, scalar1=invmn,
                                            scalar2=None, op0=Alu.mult)
                    mh[fn] = []
                    for cc in range(CC):
                        ptr = psum_tr.tile([P, K], f32, tag="ptr")
                        nc.tensor.transpose(
                            out=ptr, in_=mhT[:, cc * P:(cc + 1) * P], identity=ident19)
                        mcc = singles.tile([P, K], bf16, name=f"mh_{fn}{cc}")
                        nc.vector.tensor_copy(mcc, ptr)
                        mh[fn].append(mcc)

        # 1 / max(|feat_p|, eps) maps
        for fn in "st":
            fmap = singles.tile([P, NCH], f32, name=f"fn_{fn}")
            nc.scalar.activation(out=fmap, in_=fnsq[fn], func=Act.Sqrt)
            nc.vector.tensor_scalar_max(fmap, fmap, EPS_COS)
            nc.vector.reciprocal(invfn[fn], fmap)

        # ---------------- pass 2 ----------------
        acc = small.tile([P, 1], f32, tag="acc0")
        nc.vector.memset(acc, 0.0)
        with tc.tile_pool(name="psum2", bufs=2, space="PSUM") as psum2, \
             tc.tile_pool(name="accp", bufs=2) as accp:
            for w in range(NW):
                nats = {}
                for fn, x in (("s", xs), ("t", xt)):
                    for cc in range(CC):
                        t = nat.tile([P, WPIX], bf16, tag=f"nat_{fn}{cc}")
                        nc.gpsimd.dma_start(
                            out=t,
                            in_=x[cc * P:(cc + 1) * P, w * WPIX:(w + 1) * WPIX],
                        )
                        nats[fn, cc] = t
                gps = {}
                for fn in "st":
                    g = psum2.tile([P, CHW * K], f32, tag=f"g_{fn}")
                    for j in range(CHW):
                        for cc in range(CC):
                            nc.tensor.matmul(
                                g[:, j * K:(j + 1) * K],
                                nats[fn, cc][:, j * P:(j + 1) * P],
                                mh[fn][cc],
                                start=(cc == 0), stop=(cc == CC - 1),
                            )
                    gps[fn] = g
                dots = {}
                for fn in "st":
                    d = small.tile([P, CHW], f32, tag=f"dot_{fn}")
                    for j in range(CHW):
                        ch = w * CHW + j
                        gdum = dvetmp.tile([P, K], f32, tag="gdum")
                        nc.vector.tensor_mul(gdum, gps[fn][:, j * K:(j + 1) * K],
                                             ohT_map[:, ch * K:(ch + 1) * K])
                        nc.vector.tensor_reduce(
                            out=d[:, j:j + 1], in_=gdum,
                            axis=mybir.AxisListType.X, op=Alu.add,
                        )
                    dots[fn] = d
                cos = {}
                for fn in "st":
                    cv = small.tile([P, CHW], f32, tag=f"cos_{fn}")
                    nc.vector.tensor_mul(cv, dots[fn],
                                         invfn[fn][:, w * CHW:(w + 1) * CHW])
                    cos[fn] = cv
                diff = small.tile([P, CHW], f32, tag="diff")
                nc.vector.tensor_sub(diff, cos["s"], cos["t"])
                acc_new = accp.tile([P, 1], f32, tag="acc")
                ddum = dvetmp.tile([P, CHW], f32, tag="ddum")
                nc.vector.tensor_mul(ddum, diff, diff)
                part = small.tile([P, 1], f32, tag="part")
                nc.vector.tensor_reduce(out=part, in_=ddum,
                                        axis=mybir.AxisListType.X, op=Alu.add)
                nc.vector.tensor_add(acc_new, acc, part)
                acc = acc_new

            # ---------------- final partition reduce ----------------
            with tc.tile_pool(name="psumf", bufs=1, space="PSUM") as psumf:
                pf = psumf.tile([1, 1], f32)
                nc.tensor.matmul(pf, acc, ones_f, start=True, stop=True)
                osb = small.tile([1, 1], f32, tag="osb")
                nc.vector.tensor_copy(osb, pf)
                nc.sync.dma_start(out=o[:, :], in_=osb)

    nc.compile()
    return nc


def get_nc():
    if "nc" not in _CACHE:
        _CACHE["nc"] = _build_nc()
    return _CACHE["nc"]


def make_in_maps(preds_S, preds_T, target):
    preds_S = np.ascontiguousarray(np.asarray(preds_S, dtype=np.float32))
    preds_T = np.ascontiguousarray(np.asarray(preds_T, dtype=np.float32))
    target = np.asarray(target)
    in_maps = []
    for b in range(B):
        lab = target[b, 0].reshape(HW).astype(np.float32)
        labT = np.ascontiguousarray(lab.reshape(NCH, P).T)  # [i, ch]
        in_maps.append({
            "xs": preds_S[b].reshape(C, HW),
            "xt": preds_T[b].reshape(C, HW),
            "labT": labT,
        })
    return in_maps


def kernel(preds_S, preds_T, target):
    global LAST_RESULTS
    from concourse.bass_utils import run_bass_kernel_spmd

    nc = get_nc()
    in_maps = make_in_maps(preds_S, preds_T, target)
    res = run_bass_kernel_spmd(nc, in_maps, core_ids=list(range(B)), trace=TRACE)
    LAST_RESULTS = res
    total = np.float64(0.0)
    for r in res.results:
        total += np.float64(r["o"].reshape(-1)[0])
    return np.float32(total / (B * HW))
